# revision 24
# baseline (speedup 1.0000x reference)
"""GCN (8-layer, 16 GCNConv) on 8 TRN2 NeuronCores.

Strategy:
- dst-partition nodes across 8 cores (6250 each); weights replicated.
- norm separability: norm[e] = dis[src]*dis[dst], so each conv is
    g = dis * (h @ W)         (node-major, per-core slice)
    AllGather g (split into two half-collectives a/b)
    agg[f,d] = sum_e g_fm[src[e]] onehot[e,d]   via PE matmuls over
               128-edge chunks (msgs gathered edge-major by SWDGE dma_gather)
    h' = relu?(dis * agg + b)  (feature-major)
- self-loop folded in as one diag-matmul per 128-dst block.
- node re-packing (_pack_nodes): nodes are permuted so per-(core, stream,
  dst-block) in-edge counts fill 128-slot gather chunks tightly (degree-
  balanced cores kill collective-barrier stragglers; hot rows cluster for
  DRAM locality; padded gather slots drop ~12%). SWDGE gather cost is
  per-descriptor (~3ns/desc, elem-size-independent <=512B), so slots are
  the currency.
- software-pipelined conv loop: the epilogue + next conv's h@W/self-loop
  ("production") is interleaved into stream b's scatter loop in 4 batches,
  so each conv's AllGathers fly while the previous conv's gathers drain.
- edges are host-sorted by (stream, dst block); per-block chunk counts are
  shared across cores (max), pad slots gather row 0 with onehot id -1.
- int16 gather indices: the a/b split keeps indices < 25600.
- final mean-pool via matmul with host-built pooling matrix + AllReduce.
- tuned: SEG=1024 slots/gather-call, 16 msg/oh buffers, 4 SWDGE queues.
"""
import numpy as np
import concourse.bass as bass
import concourse.mybir as mybir
import concourse.bacc as bacc
import concourse.tile as tile
from concourse.bass_utils import run_bass_kernel_spmd

import os
N = 50000
E = 600000
D = 128
L = 8
NCONV = int(os.environ.get("GCN_NCONV", 2 * L))
SKIP_COLL = os.environ.get("GCN_SKIP_COLL", "") == "1"
SKIP_GDMA = os.environ.get("GCN_SKIP_GDMA", "") == "1"
SKIP_STREAMS = os.environ.get("GCN_SKIP_STREAMS", "") == "1"
SKIP_GATHER = os.environ.get("GCN_SKIP_GATHER", "") == "1"
DUMP_H = os.environ.get("GCN_DUMP_H", "") == "1"
C = 8
NPC = N // C              # 6250 nodes per core
NB = (NPC + 127) // 128   # 49 blocks
NPAD = NB * 128           # 6272
CH_A = 25                 # chunks 0..24 -> stream a
HALF_A = CH_A * 128       # 3200 nodes (a-half, incl none padded)
HALF_B = NPAD - HALF_A    # 3072 node slots (b-half, incl 22 pads)
NH_A = HALF_A * C         # 25600 rows in g_full_a
NH_B = HALF_B * C         # 24576 rows in g_full_b
SEG = int(os.environ.get("GCN_SEG", 1024))   # slots per dma_gather call (>1024 needs single_packet=False)
SEGC = SEG // 128         # chunks per segment
NG = 64                   # graphs

f32 = mybir.dt.float32
bf16 = mybir.dt.bfloat16
i16 = mybir.dt.int16
i32 = mybir.dt.int32
AT = mybir.AluOpType
ACTF = mybir.ActivationFunctionType


def _wrap16(vals: np.ndarray) -> np.ndarray:
    """slot i -> [i % 16, i // 16], replicated to 128 partitions."""
    n = len(vals)
    base = vals.astype(np.int16).reshape(n // 16, 16).T   # [16, n//16]
    return np.ascontiguousarray(np.tile(base, (8, 1)))


def _pack_nodes(edge_index):
    """Permute nodes so per-(core, stream, dst-block) in-edge counts pack
    tightly against 128-slot gather chunks (CPB = max over cores of
    ceil(cnt/128) -> fewer padded gather slots).

    Returns perm with perm[new_slot] = old_node (len N).
    """
    src = edge_index[0].astype(np.int64)
    dst = edge_index[1].astype(np.int64)
    indeg = np.bincount(dst, minlength=N)
    outdeg = np.bincount(src, minlength=N)

    # 1) nodes -> cores, snake by in-degree (balances per-core totals)
    order = np.argsort(-indeg, kind="stable")
    core_of = np.empty(N, np.int64)
    snake = np.tile(np.concatenate([np.arange(C), np.arange(C)[::-1]]),
                    N // (2 * C) + 1)[:N]
    core_of[order] = snake

    # 2) within core: a-group (HALF_A slots) / b-group, snake by out-degree
    in_group_a = np.zeros(N, bool)
    nodes_of_core = [np.where(core_of == c)[0] for c in range(C)]
    for c in range(C):
        nds = nodes_of_core[c]
        o = nds[np.argsort(-outdeg[nds], kind="stable")]
        sel = np.zeros(len(o), bool)
        # alternate to balance stream-a/b out-edge totals; a gets HALF_A slots
        na = HALF_A
        take = np.tile([True, False], len(o) // 2 + 1)[:len(o)]
        if take.sum() > na:
            idxs = np.where(take)[0]
            take[idxs[na:]] = False
        elif take.sum() < na:
            idxs = np.where(~take)[0]
            take[idxs[:na - int(take.sum())]] = True
        sel[:] = take
        in_group_a[o[sel]] = True

    # 3) per-node stream in-degrees given the source grouping
    src_a = in_group_a[src]
    in_a = np.bincount(dst[src_a], minlength=N)
    in_b = np.bincount(dst[~src_a], minlength=N)

    # 4) per-core greedy pack into blocks, vs a shared chunk schedule
    def schedule(tot_max, nblocks):
        q = np.full(nblocks, tot_max // nblocks // 128, np.int64)
        need = (tot_max + 127) // 128 + 2 - int(q.sum())
        q[:max(need, 0)] += 1
        return q

    # group totals per core
    ta = np.array([in_a[nodes_of_core[c][in_group_a[nodes_of_core[c]]]].sum()
                   for c in range(C)])
    tb_a = np.array([in_b[nodes_of_core[c][in_group_a[nodes_of_core[c]]]].sum()
                     for c in range(C)])
    ta_b = np.array([in_a[nodes_of_core[c][~in_group_a[nodes_of_core[c]]]].sum()
                     for c in range(C)])
    tb_b = np.array([in_b[nodes_of_core[c][~in_group_a[nodes_of_core[c]]]].sum()
                     for c in range(C)])
    qa_ga = schedule(int(ta.max()), CH_A)          # stream-a caps, blocks 0-24
    qb_ga = schedule(int(tb_a.max()), CH_A)        # stream-b caps, blocks 0-24
    qa_gb = schedule(int(ta_b.max()), NB - CH_A)   # stream-a caps, blocks 25-48
    qb_gb = schedule(int(tb_b.max()), NB - CH_A)

    perm = np.empty(N, np.int64)

    def pack(nds, qa, qb, sizes):
        """Fill bins one at a time to just under their caps. A node is
        eligible only if, after taking it, the remaining slots of this bin
        can still be filled with the smallest remaining nodes within the
        caps (suffix-sum guard). Among eligible, take the largest."""
        o = nds[np.argsort(-(in_a[nds] + in_b[nds]), kind="stable")]
        wa = in_a[o].astype(np.int64)
        wb = in_b[o].astype(np.int64)
        alive = np.ones(len(o), bool)
        assign = np.empty(len(o), np.int64)
        for j in np.argsort(-(qa + qb), kind="stable"):
            rem_a = qa[j] * 128
            rem_b = qb[j] * 128
            m = int(sizes[j])
            while m > 0:
                idxs = np.where(alive)[0]
                # sum of the (m-1) smallest remaining weights (tail of the
                # desc-sorted order) — what the rest of this bin will need
                # at minimum if we fill it with the lightest nodes
                if m > 1:
                    tail = idxs[-(m - 1):]
                    ta, tb = int(wa[tail].sum()), int(wb[tail].sum())
                else:
                    ta = tb = 0
                feas = idxs[(wa[idxs] <= rem_a - ta) & (wb[idxs] <= rem_b - tb)]
                if len(feas):
                    i = feas[0]          # largest eligible
                else:
                    i = idxs[-1]         # smallest remaining (overflow path)
                assign[i] = j
                alive[i] = False
                rem_a -= wa[i]
                rem_b -= wb[i]
                m -= 1
        return o, assign

    for c in range(C):
        nds = nodes_of_core[c]
        ga = nds[in_group_a[nds]]
        gb = nds[~in_group_a[nds]]
        # a-group blocks 0..24 all size 128
        o, asn = pack(ga, qa_ga, qb_ga, np.full(CH_A, 128, np.int64))
        slot_ctr = np.zeros(CH_A, np.int64)
        for nd, j in zip(o, asn):
            perm[c * NPC + j * 128 + slot_ctr[j]] = nd
            slot_ctr[j] += 1
        # b-group blocks 25..48; last block holds NPC - HALF_A - 23*128 real
        sizes = np.full(NB - CH_A, 128, np.int64)
        sizes[-1] = NPC - HALF_A - (NB - CH_A - 1) * 128
        o, asn = pack(gb, qa_gb, qb_gb, sizes)
        slot_ctr = np.zeros(NB - CH_A, np.int64)
        for nd, j in zip(o, asn):
            perm[c * NPC + HALF_A + j * 128 + slot_ctr[j]] = nd
            slot_ctr[j] += 1
    return perm


def host_prep(x, edge_index, batch):
    # node re-packing permutation: new slot layout
    perm = _pack_nodes(edge_index)
    inv = np.empty(N, np.int64)
    inv[perm] = np.arange(N)
    x = np.asarray(x)[perm]
    batch = np.asarray(batch)[perm]
    edge_index = inv[np.asarray(edge_index).astype(np.int64)]

    src = edge_index[0].astype(np.int64)
    dst = edge_index[1].astype(np.int64)
    deg = np.bincount(dst, minlength=N).astype(np.float64) + 1.0
    dis = (1.0 / np.sqrt(deg)).astype(np.float32)

    r = src // NPC
    k = src % NPC
    kb = k // 128
    kp = k % 128
    stream = (kb >= CH_A).astype(np.int64)              # 0 = a, 1 = b
    # chunk-major rows: a: r*3200 + p*25 + b ; b: r*3072 + p*24 + (b-25)
    loc = np.where(stream == 0,
                   r * HALF_A + kp * CH_A + kb,
                   r * HALF_B + kp * (NB - CH_A) + (kb - CH_A)).astype(np.int64)
    core_of = dst // NPC
    dloc = dst % NPC
    blk = dloc // 128
    id_in_blk = dloc % 128

    key = (core_of * 2 + stream) * NB + blk
    order = np.argsort(key, kind="stable")
    s_loc = loc[order]
    s_id = id_in_blk[order]
    gcnt = np.bincount(key, minlength=C * 2 * NB).reshape(C, 2, NB)
    goff = np.zeros(C * 2 * NB + 1, np.int64)
    np.cumsum(gcnt.reshape(-1), out=goff[1:])

    # shared chunks-per-block (max over cores), per stream
    CPB = [np.maximum.reduce(-(-gcnt[:, s, :] // 128), axis=0) for s in range(2)]
    chunk_blocks = [np.repeat(np.arange(NB), CPB[s]) for s in range(2)]
    S = [int(CPB[s].sum()) * 128 for s in range(2)]
    blk_chunk_start = [np.concatenate([[0], np.cumsum(CPB[s])]) for s in range(2)]

    idx_host = [[None] * C for _ in range(2)]
    ids_host = [[None] * C for _ in range(2)]
    for s in range(2):
        for c in range(C):
            ia = np.zeros(S[s], np.int64)
            da = np.full(S[s], -1.0, np.float32)
            for b in range(NB):
                g = (c * 2 + s) * NB + b
                cnt = goff[g + 1] - goff[g]
                o = int(blk_chunk_start[s][b]) * 128
                ia[o:o + cnt] = s_loc[goff[g]:goff[g + 1]]
                da[o:o + cnt] = s_id[goff[g]:goff[g + 1]]
            idx_host[s][c] = _wrap16(ia)
            import ml_dtypes
            ids_host[s][c] = np.ascontiguousarray(
                da.reshape(S[s] // 128, 128).T).astype(ml_dtypes.bfloat16)

    # segment calls: (slot_off, nslots, chunk0, nchunks)
    calls = []
    for s in range(2):
        cl = []
        off = 0
        while off < S[s]:
            n = min(SEG, S[s] - off)
            cl.append((off, n, off // 128, n // 128))
            off += n
        calls.append(cl)

    # per-core aux arrays
    dis_pad = np.zeros((C, NPAD), np.float32)
    for c in range(C):
        dis_pad[c, :NPC] = dis[c * NPC:(c + 1) * NPC]
    dis_nm = dis_pad.reshape(C, NB, 128).transpose(0, 2, 1).copy()   # [C,128,NB]
    dis_fm = np.repeat(dis_pad[:, None, :], 128, axis=1)             # [C,128,NPAD]

    x_fm = np.zeros((C, 128, NPAD), np.float32)
    for c in range(C):
        x_fm[c, :, :NPC] = x[c * NPC:(c + 1) * NPC].T

    cnt = np.bincount(batch.astype(np.int64), minlength=NG).astype(np.float64)
    w = (1.0 / np.maximum(cnt, 1.0)).astype(np.float32)
    pool_mat = np.zeros((C, NPAD, NG), np.float32)
    for c in range(C):
        bl = batch[c * NPC:(c + 1) * NPC].astype(np.int64)
        pool_mat[c, np.arange(NPC), bl] = w[bl]
    pool_t = pool_mat.reshape(C, NB, 128, NG).transpose(0, 2, 1, 3).reshape(
        C, 128, NB * NG).copy()

    return dict(dis_nm=dis_nm, dis_fm=dis_fm, x_fm=x_fm, pool_t=pool_t,
                idx_host=idx_host, ids_host=ids_host, calls=calls,
                CPB=CPB, chunk_blocks=chunk_blocks, S=S,
                blk_chunk_start=blk_chunk_start)


def build_program(nc, st):
    calls = st["calls"]
    CPB = st["CPB"]
    blk_start = st["blk_chunk_start"]
    S = st["S"]
    COLL_ENG = os.environ.get("GCN_COLL_ENG", "gp")

    def coll_cc(*a, **k):
        # issue collectives from the SP (sync) engine queue so they don't
        # occupy the gpsimd engine that paces SWDGE descriptor-gen
        if COLL_ENG == "sp":
            return bass.BassGpSimd.collective_compute(nc.sync, *a, **k)
        return nc.gpsimd.collective_compute(*a, **k)

    # ---- I/O ----
    x_in = nc.dram_tensor("x_fm", [128, NPAD], f32, kind="ExternalInput")
    w_in = nc.dram_tensor("wcat", [128, NCONV * 128], f32, kind="ExternalInput")
    b_in = nc.dram_tensor("bcat", [128, NCONV], f32, kind="ExternalInput")
    dnm_in = nc.dram_tensor("dis_nm", [128, NB], f32, kind="ExternalInput")
    dfm_in = nc.dram_tensor("dis_fm", [128, NPAD], f32, kind="ExternalInput")
    pool_in = nc.dram_tensor("pool_t", [128, NB * NG], f32, kind="ExternalInput")
    ident_in = nc.dram_tensor("ident", [128, 128], f32, kind="ExternalInput")
    idx_in = [nc.dram_tensor(f"idx{s}", [128, S[s] // 16], i16,
                             kind="ExternalInput") for s in range(2)]
    ids_in = [nc.dram_tensor(f"ids{s}", [128, S[s] // 128], bf16,
                             kind="ExternalInput") for s in range(2)]
    out_t = nc.dram_tensor("out", [NG, 128], f32, kind="ExternalOutput")
    hdump_t = nc.dram_tensor("hdump", [128, NPAD], f32,
                             kind="ExternalOutput") if DUMP_H else None

    g_slice = [nc.dram_tensor("g_slice0", [HALF_A, 128], bf16, kind="Internal"),
               nc.dram_tensor("g_slice1", [HALF_B, 128], bf16, kind="Internal")]
    g_full = [nc.dram_tensor("g_full0", [NH_A, 128], bf16, kind="Internal",
                             addr_space="Shared"),
              nc.dram_tensor("g_full1", [NH_B, 128], bf16, kind="Internal",
                             addr_space="Shared")]
    ar_in = nc.dram_tensor("ar_in", [NG, 128], f32, kind="Internal")
    ar_out = nc.dram_tensor("ar_out", [NG, 128], f32, kind="Internal",
                            addr_space="Shared")
    rg = [list(range(C))]

    def ap3(t, off_elems, dims):
        return bass.AP(t, off_elems, dims)

    with tile.TileContext(nc) as tc:
        with tc.tile_pool(name="const", bufs=1) as cp, \
             tc.tile_pool(name="state", bufs=1) as sp, \
             tc.tile_pool(name="ph", bufs=2, space="PSUM") as php, \
             tc.tile_pool(name="pagg", bufs=int(os.environ.get("GCN_PAGG", 5)), space="PSUM") as pap:

            b_t = cp.tile([128, NCONV], f32, tag="b")
            dnm_t = cp.tile([128, NB], f32, tag="dnm")
            dfm_t = cp.tile([128, NPAD], f32, tag="dfm")
            ident_t = cp.tile([128, 128], f32, tag="ident")
            ident_bf = cp.tile([128, 128], bf16, tag="identbf")
            iota_seg_f = cp.tile([128, SEG], bf16, tag="iosegf")
            iota_pm_f = cp.tile([128, 128], f32, tag="iopmf")

            idx_res = [cp.tile([128, S[s] // 16], i16, tag=f"idxr{s}",
                               name=f"idxr{s}") for s in range(2)]
            ids_res = [cp.tile([128, S[s] // 128], bf16, tag=f"idsr{s}",
                               name=f"idsr{s}") for s in range(2)]
            h0 = sp.tile([128, NPAD], f32, tag="h0")
            h1 = sp.tile([128, NPAD], f32, tag="h1")
            g_nm = sp.tile([128, NPAD], bf16, tag="gnm")
            hb = [h0, h1]

            nc.sync.dma_start(b_t[:], b_in[:])
            nc.sync.dma_start(dnm_t[:], dnm_in[:])
            nc.sync.dma_start(dfm_t[:], dfm_in[:])
            nc.sync.dma_start(ident_t[:], ident_in[:])
            nc.vector.tensor_copy(ident_bf[:], ident_t[:])
            nc.sync.dma_start(h0[:], x_in[:])
            for s in range(2):
                nc.sync.dma_start(idx_res[s][:], idx_in[s][:])
                nc.sync.dma_start(ids_res[s][:], ids_in[s][:])
            nc.gpsimd.iota(iota_seg_f[:], pattern=[[0, SEGC], [1, 128]],
                           base=0, channel_multiplier=0,
                           allow_small_or_imprecise_dtypes=True)
            nc.gpsimd.iota(iota_pm_f[:], pattern=[[1, 128]], base=0,
                           channel_multiplier=-1,
                           allow_small_or_imprecise_dtypes=True)

            def bs(b):
                return slice(b * 128, (b + 1) * 128)

            def emit_g_dmas(half):
                """One contiguous DMA: g_nm cols -> chunk-major slice rows."""
                if SKIP_GDMA:
                    return
                if half == 0:
                    nc.sync.dma_start(
                        ap3(g_slice[0], 0, [[HALF_A, 128], [1, HALF_A]]),
                        g_nm[:, 0:HALF_A])
                else:
                    nc.sync.dma_start(
                        ap3(g_slice[1], 0, [[HALF_B, 128], [1, HALF_B]]),
                        g_nm[:, HALF_A:NPAD])

            def emit_ag(half):
                if SKIP_COLL:
                    return
                coll_cc(
                    "AllGather", AT.bypass, replica_groups=rg,
                    ins=[g_slice[half][:]], outs=[g_full[half][:]])

            NBUF = int(os.environ.get("GCN_NBUF", 16))
            mp = tc.alloc_tile_pool(name="msg", bufs=NBUF)
            op = tc.alloc_tile_pool(name="oh", bufs=NBUF)
            tp = tc.alloc_tile_pool(name="meta", bufs=3)
            dgp = tc.alloc_tile_pool(name="dg", bufs=3)

            produced_w = {}

            def load_w(cv):
                w_t = tp.tile([128, 128], f32, tag="wt", bufs=2)
                nc.sync.dma_start(w_t[:], w_in[:, cv * 128:(cv + 1) * 128])
                produced_w[cv] = w_t

            def production(cvn, b):
                """Epilogue of conv cvn-1 for block b, then conv cvn's
                h@W + g scale + self-loop for block b. At b==24/last,
                kick the g slice DMAs for conv cvn's AllGathers."""
                cve = cvn - 1
                agg_prev = hb[cvn % 2]      # output of conv cve
                nc.vector.tensor_tensor(agg_prev[:, bs(b)], agg_prev[:, bs(b)],
                                        dfm_t[:, bs(b)], AT.mult)
                if cve % 2 == 0:
                    nc.scalar.activation(agg_prev[:, bs(b)], agg_prev[:, bs(b)],
                                         ACTF.Relu, bias=b_t[:, cve:cve + 1],
                                         scale=1.0)
                else:
                    nc.vector.tensor_scalar(agg_prev[:, bs(b)],
                                            agg_prev[:, bs(b)],
                                            b_t[:, cve:cve + 1], None, AT.add)
                if cvn >= NCONV:
                    return
                ph = php.tile([128, 128], f32, tag="ph")
                nc.tensor.matmul(ph[:], agg_prev[:, bs(b)], produced_w[cvn][:],
                                 start=True, stop=True)
                nc.scalar.activation(g_nm[:, bs(b)], ph[:], ACTF.Copy,
                                     scale=dnm_t[:, b:b + 1])
                if b == 24:
                    emit_g_dmas(0)
                if b == NB - 1:
                    emit_g_dmas(1)

            def stream_scatter(cv, s, prod_cv=None):
                agg = hb[(cv + 1) % 2]
                msg_tiles = {}
                oh_tiles = {}
                state = {"emitted": -1}

                def emit_seg(si):
                    off, n, c0, nch = calls[s][si]
                    msg = mp.tile([128, SEGC, 128], bf16, tag="msg")
                    if not SKIP_GATHER:
                        nc.gpsimd.dma_gather(
                            msg[:, :nch, :], g_full[s][:],
                            idx_res[s][:, off // 16:(off + n) // 16],
                            num_idxs=n, num_idxs_reg=n, elem_size=128,
                            single_packet=False,
                            queue_num=si % nc.num_swdge_queues)
                    else:
                        nc.vector.memset(msg[:, :nch, :], 0.0)
                    oh = op.tile([128, SEG], bf16, tag="oh")
                    sl = ids_res[s][:, c0:c0 + nch]
                    in1 = bass.AP(sl.tensor, sl.offset, sl.ap + [[0, 128]])
                    nc.vector.tensor_tensor(
                        oh[:].rearrange("p (c d) -> p c d", d=128)[:, :nch, :],
                        iota_seg_f[:].rearrange("p (c d) -> p c d", d=128)[:, :nch, :],
                        in1, AT.is_equal)
                    msg_tiles[si] = msg
                    oh_tiles[si] = oh

                import os as _os
                NBATCH = int(_os.environ.get("GCN_NBATCH", 4))
                if NBATCH == 8:
                    flush_at = {6: (0, 7), 12: (7, 13), 18: (13, 19),
                                24: (19, 25), 30: (25, 31), 36: (31, 37),
                                42: (37, 43), NB - 1: (43, NB)}
                else:
                    flush_at = {12: (0, 13), 24: (13, 25), 36: (25, 37),
                                NB - 1: (37, NB)}
                for b in range(NB):
                    nch_b = int(CPB[s][b])
                    if nch_b > 0 or s == 0:
                        pa = pap.tile([128, 128], f32, tag="pagg")
                        started = False
                        if s == 0:
                            # self-loop folded in as the first accumuland
                            nc.tensor.matmul(pa[:], g_nm[:, bs(b)],
                                             ident_bf[:], start=True,
                                             stop=(nch_b == 0))
                            started = True
                        c_lo = int(blk_start[s][b])
                        for j in range(nch_b):
                            ch = c_lo + j
                            si = ch // SEGC
                            jj = ch % SEGC
                            if si > state["emitted"]:
                                emit_seg(si)
                                state["emitted"] = si
                            nc.tensor.matmul(
                                pa[:], msg_tiles[si][:, jj, :],
                                oh_tiles[si][:, jj * 128:(jj + 1) * 128],
                                start=(not started),
                                stop=(j == nch_b - 1))
                            started = True
                        if s == 0:
                            nc.vector.tensor_copy(agg[:, bs(b)], pa[:])
                        else:
                            nc.vector.tensor_tensor(agg[:, bs(b)], pa[:],
                                                    agg[:, bs(b)], AT.add)
                    if prod_cv is not None and b in flush_at:
                        lo, hi = flush_at[b]
                        for pb in range(lo, hi):
                            production(prod_cv, pb)

            # --- prologue: conv 0's h@W + self-loop ---
            load_w(0)
            agg0 = hb[1]
            for b in range(NB):
                ph = php.tile([128, 128], f32, tag="ph")
                nc.tensor.matmul(ph[:], h0[:, bs(b)], produced_w[0][:],
                                 start=True, stop=True)
                nc.scalar.activation(g_nm[:, bs(b)], ph[:], ACTF.Copy,
                                     scale=dnm_t[:, b:b + 1])
                if b == 24:
                    emit_g_dmas(0)
                    emit_ag(0)
            emit_g_dmas(1)

            # --- pipelined conv loop ---
            for cv in range(NCONV):
                if cv + 1 < NCONV:
                    load_w(cv + 1)
                if not SKIP_STREAMS:
                    stream_scatter(cv, 0)
                    emit_ag(1)
                    stream_scatter(cv, 1, prod_cv=cv + 1)
                else:
                    emit_ag(1)
                    for b in range(NB):
                        production(cv + 1, b)
                if cv + 1 < NCONV:
                    emit_ag(0)

            for p in (dgp, tp, op, mp):
                p.release()

            # ---- mean pool + AllReduce ----
            h_fin = hb[NCONV % 2]
            if DUMP_H:
                nc.sync.dma_start(hdump_t[:], h_fin[:])
            tailp = tc.alloc_tile_pool(name="tail", bufs=1)
            pool_tile = tailp.tile([128, NB * NG], f32, tag="poolm")
            nc.sync.dma_start(pool_tile[:], pool_in[:])
            hnm = tailp.tile([128, NPAD], f32, tag="hnm")
            for b in range(NB):
                pt = php.tile([128, 128], f32, tag="ph")
                nc.tensor.transpose(pt[:], h_fin[:, bs(b)], ident_t[:])
                nc.vector.tensor_copy(hnm[:, bs(b)], pt[:])
            ppool = pap.tile([NG, 128], f32, tag="ppool", bufs=1)
            for b in range(NB):
                nc.tensor.matmul(ppool[:], pool_tile[:, b * NG:(b + 1) * NG],
                                 hnm[:, bs(b)], start=(b == 0),
                                 stop=(b == NB - 1))
            pres = sp.tile([NG, 128], f32, tag="pres")
            nc.vector.tensor_copy(pres[:], ppool[:])
            nc.sync.dma_start(ar_in[:], pres[:])
            if not SKIP_COLL:
                nc.gpsimd.collective_compute(
                    "AllReduce", AT.add, replica_groups=rg,
                    ins=[ar_in[:]], outs=[ar_out[:]])
            ores = sp.tile([NG, 128], f32, tag="ores")
            nc.sync.dma_start(ores[:], ar_out[:] if not SKIP_COLL else ar_in[:])
            nc.sync.dma_start(out_t[:], ores[:])
            tailp.release()
    return nc


def kernel(x, edge_index, batch, W1, b1, W2, b2, _want_trace=False, _want_res=False):
    x = np.asarray(x)
    edge_index = np.asarray(edge_index)
    batch = np.asarray(batch)
    W1, b1, W2, b2 = (np.asarray(a) for a in (W1, b1, W2, b2))

    st = host_prep(x, edge_index, batch)

    wcat = np.zeros((128, 2 * L * 128), np.float32)
    bcat = np.zeros((128, 2 * L), np.float32)
    for l in range(L):
        wcat[:, (2 * l) * 128:(2 * l + 1) * 128] = W1[l]
        wcat[:, (2 * l + 1) * 128:(2 * l + 2) * 128] = W2[l]
        bcat[:, 2 * l] = b1[l]
        bcat[:, 2 * l + 1] = b2[l]
    wcat = np.ascontiguousarray(wcat[:, :NCONV * 128])
    bcat = np.ascontiguousarray(bcat[:, :NCONV])

    nc = bacc.Bacc("TRN2", target_bir_lowering=False, debug=False,
                   enable_asserts=False, num_devices=C,
                   num_swdge_queues=int(os.environ.get("GCN_NQ", 4)))
    build_program(nc, st)
    nc.compile()

    ident = np.eye(128, dtype=np.float32)
    in_maps = []
    for c in range(C):
        in_maps.append({
            "x_fm": st["x_fm"][c],
            "wcat": wcat, "bcat": bcat,
            "dis_nm": st["dis_nm"][c], "dis_fm": st["dis_fm"][c],
            "pool_t": st["pool_t"][c], "ident": ident,
            "idx0": st["idx_host"][0][c], "idx1": st["idx_host"][1][c],
            "ids0": st["ids_host"][0][c], "ids1": st["ids_host"][1][c],
        })

    res = run_bass_kernel_spmd(nc, in_maps, core_ids=list(range(C)),
                               trace=_want_trace)
    out = res.results[0]["out"].astype(np.float32)
    if _want_trace or _want_res:
        return out, res
    return out



# revision 25
# speedup vs baseline: 1.2417x; 1.2417x over previous
"""GCN (8-layer, 16 GCNConv) on 8 TRN2 NeuronCores.

Strategy:
- dst-partition nodes across 8 cores (6250 each); weights replicated.
- norm separability: norm[e] = dis[src]*dis[dst], so each conv is
    g = dis * (h @ W)         (node-major, per-core slice)
    AllGather g (split into two half-collectives a/b)
    agg[f,d] = sum_e g_fm[src[e]] onehot[e,d]   via PE matmuls over
               128-edge chunks (msgs gathered edge-major by SWDGE dma_gather)
    h' = relu?(dis * agg + b)  (feature-major)
- self-loop folded in as one diag-matmul per 128-dst block.
- node re-packing (_pack_nodes): nodes are permuted so per-(core, stream,
  dst-block) in-edge counts fill 128-slot gather chunks tightly (degree-
  balanced cores kill collective-barrier stragglers; hot rows cluster for
  DRAM locality; padded gather slots drop ~12%). SWDGE gather cost is
  per-descriptor (~3ns/desc, elem-size-independent <=512B), so slots are
  the currency.
- software-pipelined conv loop: the epilogue + next conv's h@W/self-loop
  ("production") is interleaved into stream b's scatter loop in 4 batches,
  so each conv's AllGathers fly while the previous conv's gathers drain.
- edges are host-sorted by (stream, dst block); per-block chunk counts are
  shared across cores (max), pad slots gather row 0 with onehot id -1.
- int16 gather indices: the a/b split keeps indices < 25600.
- final mean-pool via matmul with host-built pooling matrix + AllReduce.
- tuned: SEG=1024 slots/gather-call, 16 msg/oh buffers, 4 SWDGE queues.
"""
import numpy as np
import concourse.bass as bass
import concourse.mybir as mybir
import concourse.bacc as bacc
import concourse.tile as tile
from concourse.bass_utils import run_bass_kernel_spmd

import os
N = 50000
E = 600000
D = 128
L = 8
NCONV = int(os.environ.get("GCN_NCONV", 2 * L))
SKIP_COLL = os.environ.get("GCN_SKIP_COLL", "") == "1"
SKIP_GDMA = os.environ.get("GCN_SKIP_GDMA", "") == "1"
SKIP_STREAMS = os.environ.get("GCN_SKIP_STREAMS", "") == "1"
SKIP_GATHER = os.environ.get("GCN_SKIP_GATHER", "") == "1"
DUMP_H = os.environ.get("GCN_DUMP_H", "") == "1"
C = 8
NPC = N // C              # 6250 nodes per core
NB = (NPC + 127) // 128   # 49 blocks
NPAD = NB * 128           # 6272
CH_A = 25                 # chunks 0..24 -> stream a
HALF_A = CH_A * 128       # 3200 nodes (a-half, incl none padded)
HALF_B = NPAD - HALF_A    # 3072 node slots (b-half, incl 22 pads)
NH_A = HALF_A * C         # 25600 rows in g_full_a
NH_B = HALF_B * C         # 24576 rows in g_full_b
SEG = int(os.environ.get("GCN_SEG", 1024))   # slots per dma_gather call (>1024 needs single_packet=False)
SEGC = SEG // 128         # chunks per segment
NG = 64                   # graphs

f32 = mybir.dt.float32
bf16 = mybir.dt.bfloat16
i16 = mybir.dt.int16
i32 = mybir.dt.int32
AT = mybir.AluOpType
ACTF = mybir.ActivationFunctionType


def _wrap16(vals: np.ndarray) -> np.ndarray:
    """slot i -> [i % 16, i // 16], replicated to 128 partitions."""
    n = len(vals)
    base = vals.astype(np.int16).reshape(n // 16, 16).T   # [16, n//16]
    return np.ascontiguousarray(np.tile(base, (8, 1)))


def _pack_nodes(edge_index):
    """Permute nodes so per-(core, stream, dst-block) in-edge counts pack
    tightly against 128-slot gather chunks (CPB = max over cores of
    ceil(cnt/128) -> fewer padded gather slots).

    Returns perm with perm[new_slot] = old_node (len N).
    """
    src = edge_index[0].astype(np.int64)
    dst = edge_index[1].astype(np.int64)
    indeg = np.bincount(dst, minlength=N)
    outdeg = np.bincount(src, minlength=N)

    # 1) nodes -> cores, snake by in-degree (balances per-core totals)
    order = np.argsort(-indeg, kind="stable")
    core_of = np.empty(N, np.int64)
    snake = np.tile(np.concatenate([np.arange(C), np.arange(C)[::-1]]),
                    N // (2 * C) + 1)[:N]
    core_of[order] = snake

    # 2) within core: a-group (HALF_A slots) / b-group, snake by out-degree
    in_group_a = np.zeros(N, bool)
    nodes_of_core = [np.where(core_of == c)[0] for c in range(C)]
    for c in range(C):
        nds = nodes_of_core[c]
        o = nds[np.argsort(-outdeg[nds], kind="stable")]
        sel = np.zeros(len(o), bool)
        # alternate to balance stream-a/b out-edge totals; a gets HALF_A slots
        na = HALF_A
        take = np.tile([True, False], len(o) // 2 + 1)[:len(o)]
        if take.sum() > na:
            idxs = np.where(take)[0]
            take[idxs[na:]] = False
        elif take.sum() < na:
            idxs = np.where(~take)[0]
            take[idxs[:na - int(take.sum())]] = True
        sel[:] = take
        in_group_a[o[sel]] = True

    # 3) per-node stream in-degrees given the source grouping
    src_a = in_group_a[src]
    in_a = np.bincount(dst[src_a], minlength=N)
    in_b = np.bincount(dst[~src_a], minlength=N)

    # 4) per-core greedy pack into blocks, vs a shared chunk schedule
    def schedule(tot_max, nblocks):
        q = np.full(nblocks, tot_max // nblocks // 128, np.int64)
        need = (tot_max + 127) // 128 + 2 - int(q.sum())
        q[:max(need, 0)] += 1
        return q

    # group totals per core
    ta = np.array([in_a[nodes_of_core[c][in_group_a[nodes_of_core[c]]]].sum()
                   for c in range(C)])
    tb_a = np.array([in_b[nodes_of_core[c][in_group_a[nodes_of_core[c]]]].sum()
                     for c in range(C)])
    ta_b = np.array([in_a[nodes_of_core[c][~in_group_a[nodes_of_core[c]]]].sum()
                     for c in range(C)])
    tb_b = np.array([in_b[nodes_of_core[c][~in_group_a[nodes_of_core[c]]]].sum()
                     for c in range(C)])
    qa_ga = schedule(int(ta.max()), CH_A)          # stream-a caps, blocks 0-24
    qb_ga = schedule(int(tb_a.max()), CH_A)        # stream-b caps, blocks 0-24
    qa_gb = schedule(int(ta_b.max()), NB - CH_A)   # stream-a caps, blocks 25-48
    qb_gb = schedule(int(tb_b.max()), NB - CH_A)

    perm = np.empty(N, np.int64)

    def pack(nds, qa, qb, sizes):
        """Fill bins one at a time to just under their caps. A node is
        eligible only if, after taking it, the remaining slots of this bin
        can still be filled with the smallest remaining nodes within the
        caps (suffix-sum guard). Among eligible, take the largest."""
        o = nds[np.argsort(-(in_a[nds] + in_b[nds]), kind="stable")]
        wa = in_a[o].astype(np.int64)
        wb = in_b[o].astype(np.int64)
        alive = np.ones(len(o), bool)
        assign = np.empty(len(o), np.int64)
        for j in np.argsort(-(qa + qb), kind="stable"):
            rem_a = qa[j] * 128
            rem_b = qb[j] * 128
            m = int(sizes[j])
            while m > 0:
                idxs = np.where(alive)[0]
                # sum of the (m-1) smallest remaining weights (tail of the
                # desc-sorted order) — what the rest of this bin will need
                # at minimum if we fill it with the lightest nodes
                if m > 1:
                    tail = idxs[-(m - 1):]
                    ta, tb = int(wa[tail].sum()), int(wb[tail].sum())
                else:
                    ta = tb = 0
                feas = idxs[(wa[idxs] <= rem_a - ta) & (wb[idxs] <= rem_b - tb)]
                if len(feas):
                    i = feas[0]          # largest eligible
                else:
                    i = idxs[-1]         # smallest remaining (overflow path)
                assign[i] = j
                alive[i] = False
                rem_a -= wa[i]
                rem_b -= wb[i]
                m -= 1
        return o, assign

    for c in range(C):
        nds = nodes_of_core[c]
        ga = nds[in_group_a[nds]]
        gb = nds[~in_group_a[nds]]
        # a-group blocks 0..24 all size 128
        o, asn = pack(ga, qa_ga, qb_ga, np.full(CH_A, 128, np.int64))
        slot_ctr = np.zeros(CH_A, np.int64)
        for nd, j in zip(o, asn):
            perm[c * NPC + j * 128 + slot_ctr[j]] = nd
            slot_ctr[j] += 1
        # b-group blocks 25..48; last block holds NPC - HALF_A - 23*128 real
        sizes = np.full(NB - CH_A, 128, np.int64)
        sizes[-1] = NPC - HALF_A - (NB - CH_A - 1) * 128
        o, asn = pack(gb, qa_gb, qb_gb, sizes)
        slot_ctr = np.zeros(NB - CH_A, np.int64)
        for nd, j in zip(o, asn):
            perm[c * NPC + HALF_A + j * 128 + slot_ctr[j]] = nd
            slot_ctr[j] += 1
    return perm


def host_prep(x, edge_index, batch):
    # node re-packing permutation: new slot layout
    perm = _pack_nodes(edge_index)
    inv = np.empty(N, np.int64)
    inv[perm] = np.arange(N)
    x = np.asarray(x)[perm]
    batch = np.asarray(batch)[perm]
    edge_index = inv[np.asarray(edge_index).astype(np.int64)]

    src = edge_index[0].astype(np.int64)
    dst = edge_index[1].astype(np.int64)
    deg = np.bincount(dst, minlength=N).astype(np.float64) + 1.0
    dis = (1.0 / np.sqrt(deg)).astype(np.float32)

    r = src // NPC
    k = src % NPC
    kb = k // 128
    kp = k % 128
    stream = (kb >= CH_A).astype(np.int64)              # 0 = a, 1 = b
    # chunk-major rows: a: r*3200 + p*25 + b ; b: r*3072 + p*24 + (b-25)
    loc = np.where(stream == 0,
                   r * HALF_A + kp * CH_A + kb,
                   r * HALF_B + kp * (NB - CH_A) + (kb - CH_A)).astype(np.int64)
    core_of = dst // NPC
    dloc = dst % NPC
    blk = dloc // 128
    id_in_blk = dloc % 128

    key = (core_of * 2 + stream) * NB + blk
    order = np.argsort(key, kind="stable")
    s_loc = loc[order]
    s_id = id_in_blk[order]
    gcnt = np.bincount(key, minlength=C * 2 * NB).reshape(C, 2, NB)
    goff = np.zeros(C * 2 * NB + 1, np.int64)
    np.cumsum(gcnt.reshape(-1), out=goff[1:])

    # shared chunks-per-block (max over cores), per stream
    CPB = [np.maximum.reduce(-(-gcnt[:, s, :] // 128), axis=0) for s in range(2)]
    chunk_blocks = [np.repeat(np.arange(NB), CPB[s]) for s in range(2)]
    S = [int(CPB[s].sum()) * 128 for s in range(2)]
    blk_chunk_start = [np.concatenate([[0], np.cumsum(CPB[s])]) for s in range(2)]

    idx_host = [[None] * C for _ in range(2)]
    ids_host = [[None] * C for _ in range(2)]
    for s in range(2):
        for c in range(C):
            ia = np.zeros(S[s], np.int64)
            da = np.full(S[s], -1.0, np.float32)
            for b in range(NB):
                g = (c * 2 + s) * NB + b
                cnt = goff[g + 1] - goff[g]
                o = int(blk_chunk_start[s][b]) * 128
                ia[o:o + cnt] = s_loc[goff[g]:goff[g + 1]]
                da[o:o + cnt] = s_id[goff[g]:goff[g + 1]]
            idx_host[s][c] = _wrap16(ia)
            import ml_dtypes
            ids_host[s][c] = np.ascontiguousarray(
                da.reshape(S[s] // 128, 128).T).astype(ml_dtypes.bfloat16)

    # segment calls: (slot_off, nslots, chunk0, nchunks)
    calls = []
    for s in range(2):
        cl = []
        off = 0
        while off < S[s]:
            n = min(SEG, S[s] - off)
            cl.append((off, n, off // 128, n // 128))
            off += n
        calls.append(cl)

    # per-core aux arrays
    dis_pad = np.zeros((C, NPAD), np.float32)
    for c in range(C):
        dis_pad[c, :NPC] = dis[c * NPC:(c + 1) * NPC]
    dis_nm = dis_pad.reshape(C, NB, 128).transpose(0, 2, 1).copy()   # [C,128,NB]
    dis_fm = np.repeat(dis_pad[:, None, :], 128, axis=1)             # [C,128,NPAD]

    x_fm = np.zeros((C, 128, NPAD), np.float32)
    for c in range(C):
        x_fm[c, :, :NPC] = x[c * NPC:(c + 1) * NPC].T

    cnt = np.bincount(batch.astype(np.int64), minlength=NG).astype(np.float64)
    w = (1.0 / np.maximum(cnt, 1.0)).astype(np.float32)
    pool_mat = np.zeros((C, NPAD, NG), np.float32)
    for c in range(C):
        bl = batch[c * NPC:(c + 1) * NPC].astype(np.int64)
        pool_mat[c, np.arange(NPC), bl] = w[bl]
    pool_t = pool_mat.reshape(C, NB, 128, NG).transpose(0, 2, 1, 3).reshape(
        C, 128, NB * NG).copy()

    return dict(dis_nm=dis_nm, dis_fm=dis_fm, x_fm=x_fm, pool_t=pool_t,
                idx_host=idx_host, ids_host=ids_host, calls=calls,
                CPB=CPB, chunk_blocks=chunk_blocks, S=S,
                blk_chunk_start=blk_chunk_start)


def build_program(nc, st):
    calls = st["calls"]
    CPB = st["CPB"]
    blk_start = st["blk_chunk_start"]
    S = st["S"]
    COLL_ENG = os.environ.get("GCN_COLL_ENG", "gp")

    def coll_cc(*a, **k):
        # issue collectives from the SP (sync) engine queue so they don't
        # occupy the gpsimd engine that paces SWDGE descriptor-gen
        if COLL_ENG == "sp":
            return bass.BassGpSimd.collective_compute(nc.sync, *a, **k)
        return nc.gpsimd.collective_compute(*a, **k)

    # ---- I/O ----
    x_in = nc.dram_tensor("x_fm", [128, NPAD], f32, kind="ExternalInput")
    w_in = nc.dram_tensor("wcat", [128, NCONV * 128], f32, kind="ExternalInput")
    b_in = nc.dram_tensor("bcat", [128, NCONV], f32, kind="ExternalInput")
    dnm_in = nc.dram_tensor("dis_nm", [128, NB], f32, kind="ExternalInput")
    dfm_in = nc.dram_tensor("dis_fm", [128, NPAD], f32, kind="ExternalInput")
    pool_in = nc.dram_tensor("pool_t", [128, NB * NG], f32, kind="ExternalInput")
    ident_in = nc.dram_tensor("ident", [128, 128], f32, kind="ExternalInput")
    idx_in = [nc.dram_tensor(f"idx{s}", [128, S[s] // 16], i16,
                             kind="ExternalInput") for s in range(2)]
    ids_in = [nc.dram_tensor(f"ids{s}", [128, S[s] // 128], bf16,
                             kind="ExternalInput") for s in range(2)]
    out_t = nc.dram_tensor("out", [NG, 128], f32, kind="ExternalOutput")
    hdump_t = nc.dram_tensor("hdump", [128, NPAD], f32,
                             kind="ExternalOutput") if DUMP_H else None

    g_slice = [nc.dram_tensor("g_slice0", [HALF_A, 128], bf16, kind="Internal"),
               nc.dram_tensor("g_slice1", [HALF_B, 128], bf16, kind="Internal")]
    g_full = [nc.dram_tensor("g_full0", [NH_A, 128], bf16, kind="Internal",
                             addr_space="Shared"),
              nc.dram_tensor("g_full1", [NH_B, 128], bf16, kind="Internal",
                             addr_space="Shared")]
    ar_in = nc.dram_tensor("ar_in", [NG, 128], f32, kind="Internal")
    ar_out = nc.dram_tensor("ar_out", [NG, 128], f32, kind="Internal",
                            addr_space="Shared")
    rg = [list(range(C))]

    def ap3(t, off_elems, dims):
        return bass.AP(t, off_elems, dims)

    with tile.TileContext(nc) as tc:
        with tc.tile_pool(name="const", bufs=1) as cp, \
             tc.tile_pool(name="state", bufs=1) as sp, \
             tc.tile_pool(name="ph", bufs=2, space="PSUM") as php, \
             tc.tile_pool(name="pagg", bufs=int(os.environ.get("GCN_PAGG", 5)), space="PSUM") as pap:

            b_t = cp.tile([128, NCONV], f32, tag="b")
            dnm_t = cp.tile([128, NB], f32, tag="dnm")
            dfm_t = cp.tile([128, NPAD], f32, tag="dfm")
            ident_t = cp.tile([128, 128], f32, tag="ident")
            ident_bf = cp.tile([128, 128], bf16, tag="identbf")
            iota_seg_f = cp.tile([128, SEG], bf16, tag="iosegf")
            iota_pm_f = cp.tile([128, 128], f32, tag="iopmf")

            idx_res = [cp.tile([128, S[s] // 16], i16, tag=f"idxr{s}",
                               name=f"idxr{s}") for s in range(2)]
            ids_res = [cp.tile([128, S[s] // 128], bf16, tag=f"idsr{s}",
                               name=f"idsr{s}") for s in range(2)]
            h0 = sp.tile([128, NPAD], f32, tag="h0")
            h1 = sp.tile([128, NPAD], f32, tag="h1")
            g_nm = sp.tile([128, NPAD], bf16, tag="gnm")
            hb = [h0, h1]

            nc.sync.dma_start(b_t[:], b_in[:])
            nc.sync.dma_start(dnm_t[:], dnm_in[:])
            nc.sync.dma_start(dfm_t[:], dfm_in[:])
            nc.sync.dma_start(ident_t[:], ident_in[:])
            nc.vector.tensor_copy(ident_bf[:], ident_t[:])
            nc.sync.dma_start(h0[:], x_in[:])
            for s in range(2):
                nc.sync.dma_start(idx_res[s][:], idx_in[s][:])
                nc.sync.dma_start(ids_res[s][:], ids_in[s][:])
            nc.gpsimd.iota(iota_seg_f[:], pattern=[[0, SEGC], [1, 128]],
                           base=0, channel_multiplier=0,
                           allow_small_or_imprecise_dtypes=True)
            nc.gpsimd.iota(iota_pm_f[:], pattern=[[1, 128]], base=0,
                           channel_multiplier=-1,
                           allow_small_or_imprecise_dtypes=True)

            def bs(b):
                return slice(b * 128, (b + 1) * 128)

            def emit_g_dmas(half):
                """One contiguous DMA: g_nm cols -> chunk-major slice rows."""
                if SKIP_GDMA:
                    return
                if half == 0:
                    nc.sync.dma_start(
                        ap3(g_slice[0], 0, [[HALF_A, 128], [1, HALF_A]]),
                        g_nm[:, 0:HALF_A])
                else:
                    nc.sync.dma_start(
                        ap3(g_slice[1], 0, [[HALF_B, 128], [1, HALF_B]]),
                        g_nm[:, HALF_A:NPAD])

            def emit_ag(half):
                if SKIP_COLL:
                    return
                coll_cc(
                    "AllGather", AT.bypass, replica_groups=rg,
                    ins=[g_slice[half][:]], outs=[g_full[half][:]])

            NBUF = int(os.environ.get("GCN_NBUF", 16))
            mp = tc.alloc_tile_pool(name="msg", bufs=NBUF)
            op = tc.alloc_tile_pool(name="oh", bufs=NBUF)
            tp = tc.alloc_tile_pool(name="meta", bufs=3)
            dgp = tc.alloc_tile_pool(name="dg", bufs=3)

            produced_w = {}

            def load_w(cv):
                w_t = tp.tile([128, 128], f32, tag="wt", bufs=2)
                nc.sync.dma_start(w_t[:], w_in[:, cv * 128:(cv + 1) * 128])
                produced_w[cv] = w_t

            def production(cvn, b):
                """Epilogue of conv cvn-1 for block b, then conv cvn's
                h@W + g scale + self-loop for block b. At b==24/last,
                kick the g slice DMAs for conv cvn's AllGathers."""
                cve = cvn - 1
                agg_prev = hb[cvn % 2]      # output of conv cve
                nc.vector.tensor_tensor(agg_prev[:, bs(b)], agg_prev[:, bs(b)],
                                        dfm_t[:, bs(b)], AT.mult)
                if cve % 2 == 0:
                    nc.scalar.activation(agg_prev[:, bs(b)], agg_prev[:, bs(b)],
                                         ACTF.Relu, bias=b_t[:, cve:cve + 1],
                                         scale=1.0)
                else:
                    nc.vector.tensor_scalar(agg_prev[:, bs(b)],
                                            agg_prev[:, bs(b)],
                                            b_t[:, cve:cve + 1], None, AT.add)
                if cvn >= NCONV:
                    return
                agg_next = hb[(cvn + 1) % 2]
                ph = php.tile([128, 128], f32, tag="ph")
                nc.tensor.matmul(ph[:], agg_prev[:, bs(b)], produced_w[cvn][:],
                                 start=True, stop=True)
                nc.scalar.activation(g_nm[:, bs(b)], ph[:], ACTF.Copy,
                                     scale=dnm_t[:, b:b + 1])
                psl = pap.tile([128, 128], f32, tag="pagg", name="psl")
                nc.tensor.matmul(psl[:], g_nm[:, bs(b)], ident_bf[:],
                                 start=True, stop=True)
                nc.vector.tensor_copy(agg_next[:, bs(b)], psl[:])
                if b == 24:
                    emit_g_dmas(0)
                if b == NB - 1:
                    emit_g_dmas(1)

            def stream_scatter(cv, s, prod_cv=None):
                agg = hb[(cv + 1) % 2]
                msg_tiles = {}
                oh_tiles = {}
                state = {"emitted": -1}

                def emit_seg(si):
                    off, n, c0, nch = calls[s][si]
                    msg = mp.tile([128, SEGC, 128], bf16, tag="msg")
                    if not SKIP_GATHER:
                        nc.gpsimd.dma_gather(
                            msg[:, :nch, :], g_full[s][:],
                            idx_res[s][:, off // 16:(off + n) // 16],
                            num_idxs=n, num_idxs_reg=n, elem_size=128,
                            single_packet=False,
                            queue_num=si % nc.num_swdge_queues)
                    else:
                        nc.vector.memset(msg[:, :nch, :], 0.0)
                    oh = op.tile([128, SEG], bf16, tag="oh")
                    sl = ids_res[s][:, c0:c0 + nch]
                    in1 = bass.AP(sl.tensor, sl.offset, sl.ap + [[0, 128]])
                    nc.vector.tensor_tensor(
                        oh[:].rearrange("p (c d) -> p c d", d=128)[:, :nch, :],
                        iota_seg_f[:].rearrange("p (c d) -> p c d", d=128)[:, :nch, :],
                        in1, AT.is_equal)
                    msg_tiles[si] = msg
                    oh_tiles[si] = oh

                import os as _os
                NBATCH = int(_os.environ.get("GCN_NBATCH", 4))
                if NBATCH == 8:
                    flush_at = {6: (0, 7), 12: (7, 13), 18: (13, 19),
                                24: (19, 25), 30: (25, 31), 36: (31, 37),
                                42: (37, 43), NB - 1: (43, NB)}
                else:
                    flush_at = {12: (0, 13), 24: (13, 25), 36: (25, 37),
                                NB - 1: (37, NB)}
                for b in range(NB):
                    nch_b = int(CPB[s][b])
                    if nch_b > 0:
                        pa = pap.tile([128, 128], f32, tag="pagg")
                        started = False
                        c_lo = int(blk_start[s][b])
                        for j in range(nch_b):
                            ch = c_lo + j
                            si = ch // SEGC
                            jj = ch % SEGC
                            if si > state["emitted"]:
                                emit_seg(si)
                                state["emitted"] = si
                            nc.tensor.matmul(
                                pa[:], msg_tiles[si][:, jj, :],
                                oh_tiles[si][:, jj * 128:(jj + 1) * 128],
                                start=(not started),
                                stop=(j == nch_b - 1))
                            started = True
                        nc.vector.tensor_tensor(agg[:, bs(b)], pa[:],
                                                agg[:, bs(b)], AT.add)
                    if prod_cv is not None and b in flush_at:
                        lo, hi = flush_at[b]
                        for pb in range(lo, hi):
                            production(prod_cv, pb)

            # --- prologue: conv 0's h@W + self-loop ---
            load_w(0)
            agg0 = hb[1]
            for b in range(NB):
                ph = php.tile([128, 128], f32, tag="ph")
                nc.tensor.matmul(ph[:], h0[:, bs(b)], produced_w[0][:],
                                 start=True, stop=True)
                nc.scalar.activation(g_nm[:, bs(b)], ph[:], ACTF.Copy,
                                     scale=dnm_t[:, b:b + 1])
                psl = pap.tile([128, 128], f32, tag="pagg", name="psl")
                nc.tensor.matmul(psl[:], g_nm[:, bs(b)], ident_bf[:],
                                 start=True, stop=True)
                nc.vector.tensor_copy(agg0[:, bs(b)], psl[:])
                if b == 24:
                    emit_g_dmas(0)
                    emit_ag(0)
            emit_g_dmas(1)

            # --- pipelined conv loop ---
            for cv in range(NCONV):
                if cv + 1 < NCONV:
                    load_w(cv + 1)
                if not SKIP_STREAMS:
                    stream_scatter(cv, 0)
                    emit_ag(1)
                    stream_scatter(cv, 1, prod_cv=cv + 1)
                else:
                    emit_ag(1)
                    for b in range(NB):
                        production(cv + 1, b)
                if cv + 1 < NCONV:
                    emit_ag(0)

            for p in (dgp, tp, op, mp):
                p.release()

            # ---- mean pool + AllReduce ----
            h_fin = hb[NCONV % 2]
            if DUMP_H:
                nc.sync.dma_start(hdump_t[:], h_fin[:])
            tailp = tc.alloc_tile_pool(name="tail", bufs=1)
            pool_tile = tailp.tile([128, NB * NG], f32, tag="poolm")
            nc.sync.dma_start(pool_tile[:], pool_in[:])
            hnm = tailp.tile([128, NPAD], f32, tag="hnm")
            for b in range(NB):
                pt = php.tile([128, 128], f32, tag="ph")
                nc.tensor.transpose(pt[:], h_fin[:, bs(b)], ident_t[:])
                nc.vector.tensor_copy(hnm[:, bs(b)], pt[:])
            ppool = pap.tile([NG, 128], f32, tag="ppool", bufs=1)
            for b in range(NB):
                nc.tensor.matmul(ppool[:], pool_tile[:, b * NG:(b + 1) * NG],
                                 hnm[:, bs(b)], start=(b == 0),
                                 stop=(b == NB - 1))
            pres = sp.tile([NG, 128], f32, tag="pres")
            nc.vector.tensor_copy(pres[:], ppool[:])
            nc.sync.dma_start(ar_in[:], pres[:])
            if not SKIP_COLL:
                nc.gpsimd.collective_compute(
                    "AllReduce", AT.add, replica_groups=rg,
                    ins=[ar_in[:]], outs=[ar_out[:]])
            ores = sp.tile([NG, 128], f32, tag="ores")
            nc.sync.dma_start(ores[:], ar_out[:] if not SKIP_COLL else ar_in[:])
            nc.sync.dma_start(out_t[:], ores[:])
            tailp.release()
    return nc


def kernel(x, edge_index, batch, W1, b1, W2, b2, _want_trace=False, _want_res=False):
    x = np.asarray(x)
    edge_index = np.asarray(edge_index)
    batch = np.asarray(batch)
    W1, b1, W2, b2 = (np.asarray(a) for a in (W1, b1, W2, b2))

    st = host_prep(x, edge_index, batch)

    wcat = np.zeros((128, 2 * L * 128), np.float32)
    bcat = np.zeros((128, 2 * L), np.float32)
    for l in range(L):
        wcat[:, (2 * l) * 128:(2 * l + 1) * 128] = W1[l]
        wcat[:, (2 * l + 1) * 128:(2 * l + 2) * 128] = W2[l]
        bcat[:, 2 * l] = b1[l]
        bcat[:, 2 * l + 1] = b2[l]
    wcat = np.ascontiguousarray(wcat[:, :NCONV * 128])
    bcat = np.ascontiguousarray(bcat[:, :NCONV])

    nc = bacc.Bacc("TRN2", target_bir_lowering=False, debug=False,
                   enable_asserts=False, num_devices=C,
                   num_swdge_queues=int(os.environ.get("GCN_NQ", 4)))
    build_program(nc, st)
    nc.compile()

    ident = np.eye(128, dtype=np.float32)
    in_maps = []
    for c in range(C):
        in_maps.append({
            "x_fm": st["x_fm"][c],
            "wcat": wcat, "bcat": bcat,
            "dis_nm": st["dis_nm"][c], "dis_fm": st["dis_fm"][c],
            "pool_t": st["pool_t"][c], "ident": ident,
            "idx0": st["idx_host"][0][c], "idx1": st["idx_host"][1][c],
            "ids0": st["ids_host"][0][c], "ids1": st["ids_host"][1][c],
        })

    res = run_bass_kernel_spmd(nc, in_maps, core_ids=list(range(C)),
                               trace=_want_trace)
    out = res.results[0]["out"].astype(np.float32)
    if _want_trace or _want_res:
        return out, res
    return out

